# revision 11
# baseline (speedup 1.0000x reference)
"""Trainium2 Bass kernel for the 3-layer LSTM highway encoder.

Problem: nn_Encoding_layer (B=32, T=512, D=H=512)
  net = lstm1(x)                          # forward LSTM
  net = hw2(net)   = rev-LSTM + highway   # reversed LSTM (per-length) + highway
  net = hw3(net)   = fwd-LSTM + highway

Sharding: data-parallel, batch 32 -> 8 cores x 4 sequences. Weights replicated.

Device design (per core, everything SBUF-resident, bf16 matmuls / fp32 state):
  - Activations live transposed: [128 d-partitions, 4 d-chunks, PAD+T*4+PAD cols]
    column index = t*4 + b. Zero pads at both ends make the t=0 (forward) and
    t=T-1 (backward) steps read zero recurrent state with no special cases.
  - Phase A (per layer): xg = x @ Wx + b as 256 dense matmuls (Wx stationary
    tiles, activations moving), bias folded via ACT Identity, output bf16
    "xgt" [128, 16 gate-chunks, T*4], gate-chunk order is group/gate-major.
  - Phase B (per layer): T sequential steps in a For_i hardware loop.
    Per step: 64 matmuls (Wh [128,128] bf16 stationary tiles, moving h_{t-1}
    [128,4]) accumulating into per-group PSUM; vector tail computes
    c = sig(f)*c + sig(i)*tanh(j), h = sig(o)*tanh(c) and writes h into the
    layer output buffer (which doubles as next step's matmul input).
    Layer 2 runs t backwards and multiplies c by a host-built (t<len) mask,
    which reproduces tf.reverse_sequence + dynamic_rnn masking exactly.
  - Phase C (layers 2,3): highway gate tg = sigmoid([prev_h, x] @ Wt + bt),
    out = y*tg + (x@Wc)*(1-tg). prev_h is just a 4-column-shifted slice.
  - Final (t >= len) output masking is done on host.
"""

import os

import ml_dtypes
import numpy as np

BF16 = ml_dtypes.bfloat16

# ---------------------------------------------------------------- constants
B, D, H = 32, 512, 512
T = int(os.environ.get("BASSLSTM_T", "512"))
NCORES = 8
BC = B // NCORES            # 4 sequences per core
P = 128
KC = D // P                 # 4 d-chunks
GC = 4 * H // P             # 16 gate chunks
NG = 2                      # hidden-chunk groups in the recurrence tail
S = KC // NG                # hidden chunks per group
GPG = GC // NG              # gate chunks per group
PAD = BC
TB = T * BC                 # columns per d-chunk
PADT = PAD + TB + PAD
NSZ = min(512, TB)          # moving free-dim per phase-A/C matmul
NCH = TB // NSZ             # tb chunks
U = 16 if T % 16 == 0 else 4  # time unroll inside For_i

# g' (gate-chunk) order: groups of hidden chunks, gate-major inside a group:
# [j j .. | i i .. | f f .. | o o ..] per group.  orig TF gate order: i,j,f,o.
_GATES = (1, 0, 2, 3)       # j, i, f, o -> index into the 4H axis


def _gprime_table():
    tbl = []
    for g in range(NG):
        for go in _GATES:
            for s in range(S):
                tbl.append((go, g * S + s))
    return tbl


# ---------------------------------------------------------------- program
_PROG = None


def _build_program():
    import concourse.mybir as mybir
    import concourse.tile as tile
    from concourse import bacc
    from concourse.bass import ds

    F32 = mybir.dt.float32
    BF = mybir.dt.bfloat16
    AF = mybir.ActivationFunctionType
    OP = mybir.AluOpType

    nc = bacc.Bacc("TRN2", target_bir_lowering=False, debug=False,
                   num_devices=NCORES)

    x0_d = nc.dram_tensor("x0t", [P, KC, PADT], BF, kind="ExternalInput")
    wx_d = nc.dram_tensor("wx", [3, P, KC, GC, P], BF, kind="ExternalInput")
    wh_d = nc.dram_tensor("wh", [3, P, KC, GC, P], BF, kind="ExternalInput")
    wt_d = nc.dram_tensor("wt", [2, P, 2 * KC, KC, P], BF, kind="ExternalInput")
    wc_d = nc.dram_tensor("wc", [2, P, KC, KC, P], BF, kind="ExternalInput")
    bias_d = nc.dram_tensor("bias", [3, P, GC], F32, kind="ExternalInput")
    bt_d = nc.dram_tensor("bt", [2, P, KC], F32, kind="ExternalInput")
    mask_d = nc.dram_tensor("mask2", [P, T * KC * BC], BF, kind="ExternalInput")
    out_d = nc.dram_tensor("outt", [P, KC, TB], F32, kind="ExternalOutput")

    with tile.TileContext(nc) as tc:
        with (
            tc.tile_pool(name="per", bufs=1) as per,
            tc.tile_pool(name="wpool", bufs=1) as wpool,
            tc.tile_pool(name="work", bufs=3) as work,
            tc.tile_pool(name="hwork", bufs=3) as hwork,
            tc.tile_pool(name="psb", bufs=4, space="PSUM") as psb,
            tc.tile_pool(name="psbig", bufs=3, space="PSUM") as psbig,
        ):
            buf1 = per.tile([P, KC, PADT], BF)
            buf2 = per.tile([P, KC, PADT], BF)
            buf3 = per.tile([P, KC, PADT], BF)
            xgt = per.tile([P, GC, TB], BF)
            biasb = per.tile([P, 3 * GC], F32)
            btb = per.tile([P, 2 * KC], F32)
            maskb = per.tile([P, T * KC * BC], BF)
            c = per.tile([P, KC, BC], F32)

            # initial loads
            nc.sync.dma_start(buf1[:], x0_d[:])
            for l in range(3):
                nc.sync.dma_start(biasb[:, l * GC:(l + 1) * GC], bias_d[l])
            for li in range(2):
                nc.sync.dma_start(btb[:, li * KC:(li + 1) * KC], bt_d[li])
            nc.sync.dma_start(maskb[:], mask_d[:])
            # zero pads of the two reusable buffers (buf1 pads come from host)
            for buf in (buf2, buf3):
                nc.vector.memset(buf[:, :, 0:PAD], 0.0)
                nc.vector.memset(buf[:, :, PAD + TB:], 0.0)

            def load_w(pool, src, shape, tag):
                t_ = pool.tile(shape, BF, tag=tag)
                nc.sync.dma_start(t_[:], src)
                return t_

            def phase_xg(l, xin, wxb):
                for gp in range(GC):
                    for n in range(NCH):
                        psum = psbig.tile([P, NSZ], F32, tag="big")
                        for k in range(KC):
                            nc.tensor.matmul(
                                psum[:], wxb[:, k, gp, :],
                                xin[:, k, PAD + n * NSZ:PAD + (n + 1) * NSZ],
                                start=(k == 0), stop=(k == KC - 1))
                        nc.scalar.activation(
                            xgt[:, gp, n * NSZ:(n + 1) * NSZ], psum[:],
                            AF.Identity, bias=biasb[:, l * GC + gp:l * GC + gp + 1])

            def mm_order():
                first_ks = list(range(KC // 2))
                rest_ks = list(range(KC // 2, KC))
                order = [(gp, k) for k in first_ks for gp in range(GC)]
                for g in range(NG):
                    order += [(gp, k)
                              for gp in range(g * GPG, (g + 1) * GPG)
                              for k in rest_ks]
                return order

            MM_ORDER = mm_order()

            GW = KC * BC  # mask columns per time step

            def phase_rec(l, yout, whb, rev):
                nc.vector.memset(c[:], 0.0)
                with tc.For_i(0, T // U, 1,
                              hint_engines=(mybir.EngineType.PE,)) as i:
                    for u in range(U):
                        if not rev:
                            # t = i*U + u
                            hcol = i * (BC * U) + u * BC            # PAD+(t-1)*BC
                            tcol = i * (BC * U) + u * BC            # t*BC
                            wcol = i * (BC * U) + PAD + u * BC      # PAD+t*BC
                            mcol = i * (GW * U) + u * GW            # t*GW
                        else:
                            # t = T-1 - (i*U + u)
                            hcol = i * (-BC * U) + PAD + (T - u) * BC
                            tcol = i * (-BC * U) + (T - 1 - u) * BC
                            wcol = i * (-BC * U) + PAD + (T - 1 - u) * BC
                            mcol = i * (-GW * U) + (T - 1 - u) * GW
                        pss = [psb.tile([P, GPG, BC], F32, tag="psb",
                                        name=f"ps{g_}")
                               for g_ in range(NG)]
                        for gp, k in MM_ORDER:
                            g, gl = divmod(gp, GPG)
                            # start=True clears has_written for the WHOLE bank,
                            # so only the first matmul into each psum tile may
                            # set it; other slices' first write (k==0) then
                            # overwrites (bit clear) and k>0 accumulates.
                            nc.tensor.matmul(
                                pss[g][:, gl, :], whb[:, k, gp, :],
                                yout[:, k, ds(hcol, BC)],
                                start=(k == 0 and gl == 0),
                                stop=(k == KC - 1),
                                skip_group_check=True)
                        for g in range(NG):
                            gsb = work.tile([P, 4 * S, BC], F32, tag="gsb")
                            nc.vector.tensor_tensor(
                                gsb[:], pss[g][:],
                                xgt[:, g * GPG:(g + 1) * GPG, ds(tcol, BC)],
                                OP.add)
                            tj = work.tile([P, S, BC], F32, tag="tj")
                            nc.scalar.activation(tj[:], gsb[:, 0:S, :], AF.Tanh)
                            sio = work.tile([P, 3 * S, BC], F32, tag="sio")
                            nc.scalar.activation(sio[:], gsb[:, S:4 * S, :],
                                                 AF.Sigmoid)
                            t1 = work.tile([P, S, BC], F32, tag="t1")
                            nc.vector.tensor_tensor(t1[:], sio[:, 0:S, :],
                                                    tj[:], OP.mult)
                            cg = c[:, g * S:(g + 1) * S, :]
                            nc.vector.tensor_tensor(cg, sio[:, S:2 * S, :],
                                                    cg, OP.mult)
                            nc.vector.tensor_tensor(cg, cg, t1[:], OP.add)
                            if rev:
                                msl = maskb[:, ds(mcol + g * S * BC, S * BC)]
                                nc.vector.tensor_tensor(
                                    cg, cg,
                                    msl.rearrange("p (a b) -> p a b", b=BC),
                                    OP.mult)
                            tct = work.tile([P, S, BC], F32, tag="tct")
                            nc.scalar.activation(tct[:], cg, AF.Tanh)
                            nc.vector.tensor_tensor(
                                yout[:, g * S:(g + 1) * S, ds(wcol, BC)],
                                sio[:, 2 * S:3 * S, :], tct[:], OP.mult)

            def phase_hw(li, y, x, out_sbuf, wtb, wcb):
                for gc_ in range(KC):
                    for n in range(NCH):
                        pt = psbig.tile([P, NSZ], F32, tag="big")
                        for k in range(KC):
                            nc.tensor.matmul(
                                pt[:], wtb[:, k, gc_, :],
                                y[:, k, PAD + n * NSZ - BC:PAD + (n + 1) * NSZ - BC],
                                start=(k == 0), stop=False)
                        for k in range(KC):
                            nc.tensor.matmul(
                                pt[:], wtb[:, KC + k, gc_, :],
                                x[:, k, PAD + n * NSZ:PAD + (n + 1) * NSZ],
                                start=False, stop=(k == KC - 1))
                        tg = hwork.tile([P, NSZ], BF, tag="tg")
                        nc.scalar.activation(
                            tg[:], pt[:], AF.Sigmoid,
                            bias=btb[:, li * KC + gc_:li * KC + gc_ + 1])
                        pc = psbig.tile([P, NSZ], F32, tag="big")
                        for k in range(KC):
                            nc.tensor.matmul(
                                pc[:], wcb[:, k, gc_, :],
                                x[:, k, PAD + n * NSZ:PAD + (n + 1) * NSZ],
                                start=(k == 0), stop=(k == KC - 1))
                        dt_ = hwork.tile([P, NSZ], F32, tag="dt")
                        nc.vector.tensor_tensor(
                            dt_[:], y[:, gc_, PAD + n * NSZ:PAD + (n + 1) * NSZ],
                            pc[:], OP.subtract)
                        nc.vector.tensor_tensor(dt_[:], dt_[:], tg[:], OP.mult)
                        if out_sbuf is not None:
                            nc.vector.tensor_tensor(
                                out_sbuf[:, gc_, PAD + n * NSZ:PAD + (n + 1) * NSZ],
                                dt_[:], pc[:], OP.add)
                        else:
                            st = hwork.tile([P, NSZ], F32, tag="st")
                            nc.vector.tensor_tensor(st[:], dt_[:], pc[:], OP.add)
                            nc.sync.dma_start(
                                out_d[:, gc_, n * NSZ:(n + 1) * NSZ], st[:])

            NL = int(os.environ.get("BASSLSTM_LAYERS", "3"))

            # ---- layer 1 (plain forward LSTM)
            wxb = load_w(wpool, wx_d[0], [P, KC, GC, P], "wx")
            whb = load_w(wpool, wh_d[0], [P, KC, GC, P], "wh")
            phase_xg(0, buf1, wxb)
            phase_rec(0, buf2, whb, rev=False)
            if NL == 1:
                for gc_ in range(KC):
                    for n in range(NCH):
                        st0 = hwork.tile([P, NSZ], F32, tag="st", name="st0")
                        nc.vector.tensor_copy(
                            st0[:],
                            buf2[:, gc_, PAD + n * NSZ:PAD + (n + 1) * NSZ])
                        nc.sync.dma_start(
                            out_d[:, gc_, n * NSZ:(n + 1) * NSZ], st0[:])
            else:
                wxb2 = load_w(wpool, wx_d[1], [P, KC, GC, P], "wx")
                wtb = load_w(wpool, wt_d[0], [P, 2 * KC, KC, P], "wt")
                wcb = load_w(wpool, wc_d[0], [P, KC, KC, P], "wc")
                whb2 = load_w(wpool, wh_d[1], [P, KC, GC, P], "wh")

                # ---- layer 2 (reversed LSTM + highway)
                phase_xg(1, buf2, wxb2)
                phase_rec(1, buf3, whb2, rev=True)
                if NL == 2:
                    phase_hw(0, buf3, buf2, None, wtb, wcb)
                else:
                    wxb3 = load_w(wpool, wx_d[2], [P, KC, GC, P], "wx")
                    whb3 = load_w(wpool, wh_d[2], [P, KC, GC, P], "wh")
                    phase_hw(0, buf3, buf2, buf1, wtb, wcb)
                    wtb2 = load_w(wpool, wt_d[1], [P, 2 * KC, KC, P], "wt")
                    wcb2 = load_w(wpool, wc_d[1], [P, KC, KC, P], "wc")

                    # ---- layer 3 (forward LSTM + highway -> DRAM)
                    phase_xg(2, buf1, wxb3)
                    phase_rec(2, buf2, whb3, rev=False)
                    phase_hw(1, buf2, buf1, None, wtb2, wcb2)

    nc.compile()
    return nc


def _program():
    global _PROG
    if _PROG is None:
        _PROG = _build_program()
    return _PROG


# ---------------------------------------------------------------- host side
def _prep_weights(inp):
    """Build the shared (replicated) weight arrays in device layout."""
    gtbl = _gprime_table()
    wx = np.zeros((3, P, KC, GC, P), np.float32)
    wh = np.zeros((3, P, KC, GC, P), np.float32)
    bias = np.zeros((3, P, GC), np.float32)
    for l, (wxn, whn, bn) in enumerate(
            [("Wx1", "Wh1", "b1"), ("Wx2", "Wh2", "b2"), ("Wx3", "Wh3", "b3")]):
        Wx = np.asarray(inp[wxn], np.float32)
        Wh = np.asarray(inp[whn], np.float32)
        b = np.asarray(inp[bn], np.float32)
        for gp, (go, m) in enumerate(gtbl):
            cs = go * H + m * P
            for k in range(KC):
                wx[l, :, k, gp, :] = Wx[k * P:(k + 1) * P, cs:cs + P]
                wh[l, :, k, gp, :] = Wh[k * P:(k + 1) * P, cs:cs + P]
            bias[l, :, gp] = b[cs:cs + P]
            if go == 2:  # forget gate: fold forget_bias = 1.0
                bias[l, :, gp] += 1.0
    wt = np.zeros((2, P, 2 * KC, KC, P), np.float32)
    wc = np.zeros((2, P, KC, KC, P), np.float32)
    bt = np.zeros((2, P, KC), np.float32)
    for li, (wtn, wcn, btn) in enumerate(
            [("Wt2", "Wc2", "bt2"), ("Wt3", "Wc3", "bt3")]):
        Wt = np.asarray(inp[wtn], np.float32)
        Wc = np.asarray(inp[wcn], np.float32)
        btv = np.asarray(inp[btn], np.float32)
        for gc_ in range(KC):
            cs = gc_ * P
            for k in range(2 * KC):
                wt[li, :, k, gc_, :] = Wt[k * P:(k + 1) * P, cs:cs + P]
            for k in range(KC):
                wc[li, :, k, gc_, :] = Wc[k * P:(k + 1) * P, cs:cs + P]
            bt[li, :, gc_] = btv[cs:cs + P]
    return (wx.astype(BF16), wh.astype(BF16), wt.astype(BF16),
            wc.astype(BF16), bias, bt)


def _host_prep(inputs):
    x = np.asarray(inputs["inputs"], np.float32)
    length = np.asarray(inputs["length"], np.int32)
    wx, wh, wt, wc, bias, bt = _prep_weights(inputs)
    in_maps = []
    for ci in range(NCORES):
        xc = x[ci * BC:(ci + 1) * BC, :T]          # [BC, T, D]
        arr = np.ascontiguousarray(xc.transpose(2, 1, 0))  # [D, T, BC]
        x0t = np.zeros((P, KC, PADT), BF16)
        x0t[:, :, PAD:PAD + TB] = (
            arr.reshape(KC, P, TB).transpose(1, 0, 2).astype(BF16))
        lc = length[ci * BC:(ci + 1) * BC]
        m = (np.arange(T)[:, None] < lc[None, :]).astype(np.float32)  # [T, BC]
        m4 = np.broadcast_to(m[:, None, :], (T, KC, BC))   # same for each chunk
        mask2 = np.ascontiguousarray(
            np.broadcast_to(m4.reshape(1, -1), (P, T * KC * BC))).astype(BF16)
        in_maps.append({
            "x0t": x0t,
            "wx": wx, "wh": wh, "wt": wt, "wc": wc,
            "bias": bias, "bt": bt,
            "mask2": mask2,
        })
    return in_maps


def _host_post(results, inputs):
    length = np.asarray(inputs["length"], np.int32)
    out = np.zeros((B, T, D), np.float32)
    for ci, res in enumerate(results):
        o = res["outt"]                      # [P, KC, TB]
        o = o.reshape(P, KC, T, BC).transpose(3, 2, 1, 0)  # [BC, T, KC, P]
        out[ci * BC:(ci + 1) * BC] = o.reshape(BC, T, D)
    tmask = np.arange(T)[None, :] < length[:, None]
    out *= tmask[:, :, None]
    return out


def run_device(in_maps):
    from concourse.bass_utils import run_bass_kernel_spmd
    nc = _program()
    res = run_bass_kernel_spmd(nc, in_maps, core_ids=list(range(NCORES)))
    return res.results


def kernel(**inputs):
    in_maps = _host_prep(inputs)
    results = run_device(in_maps)
    return _host_post(results, inputs)


# revision 12
# speedup vs baseline: 15.5745x; 15.5745x over previous
"""Trainium2 Bass kernel for the 3-layer LSTM highway encoder.

Problem: nn_Encoding_layer (B=32, T=512, D=H=512)
  net = lstm1(x)                          # forward LSTM
  net = hw2(net)   = rev-LSTM + highway   # reversed LSTM (per-length) + highway
  net = hw3(net)   = fwd-LSTM + highway

Sharding: data-parallel, batch 32 -> 8 cores x 4 sequences. Weights replicated.

Device design (per core, everything SBUF-resident, bf16 matmuls / fp32 state):
  - Activations live transposed: [128 d-partitions, 4 d-chunks, PAD+T*4+PAD cols]
    column index = t*4 + b. Zero pads at both ends make the t=0 (forward) and
    t=T-1 (backward) steps read zero recurrent state with no special cases.
  - Phase A (per layer): xg = x @ Wx + b as 256 dense matmuls (Wx stationary
    tiles, activations moving), bias folded via ACT Identity, output bf16
    "xgt" [128, 16 gate-chunks, T*4], gate-chunk order is group/gate-major.
  - Phase B (per layer): T sequential steps in a For_i hardware loop.
    Per step: 64 matmuls (Wh [128,128] bf16 stationary tiles, moving h_{t-1}
    [128,4]) accumulating into per-group PSUM; vector tail computes
    c = sig(f)*c + sig(i)*tanh(j), h = sig(o)*tanh(c) and writes h into the
    layer output buffer (which doubles as next step's matmul input).
    Layer 2 runs t backwards and multiplies c by a host-built (t<len) mask,
    which reproduces tf.reverse_sequence + dynamic_rnn masking exactly.
  - Phase C (layers 2,3): highway gate tg = sigmoid([prev_h, x] @ Wt + bt),
    out = y*tg + (x@Wc)*(1-tg). prev_h is just a 4-column-shifted slice.
  - Final (t >= len) output masking is done on host.
"""

import os

import ml_dtypes
import numpy as np

BF16 = ml_dtypes.bfloat16

# ---------------------------------------------------------------- constants
B, D, H = 32, 512, 512
T = int(os.environ.get("BASSLSTM_T", "512"))
NCORES = 8
BC = B // NCORES            # 4 sequences per core
P = 128
KC = D // P                 # 4 d-chunks
GC = 4 * H // P             # 16 gate chunks
NG = 2                      # hidden-chunk groups in the recurrence tail
S = KC // NG                # hidden chunks per group
GPG = GC // NG              # gate chunks per group
PAD = BC
TB = T * BC                 # columns per d-chunk
PADT = PAD + TB + PAD
NSZ = min(512, TB)          # moving free-dim per phase-A/C matmul
NCH = TB // NSZ             # tb chunks
U = 16 if T % 16 == 0 else 4  # time unroll inside For_i

# g' (gate-chunk) order: groups of hidden chunks, gate-major inside a group:
# [j j .. | i i .. | f f .. | o o ..] per group.  orig TF gate order: i,j,f,o.
_GATES = (1, 0, 2, 3)       # j, i, f, o -> index into the 4H axis


def _gprime_table():
    tbl = []
    for g in range(NG):
        for go in _GATES:
            for s in range(S):
                tbl.append((go, g * S + s))
    return tbl


# ---------------------------------------------------------------- program
_PROG = None


def _build_program():
    import concourse.mybir as mybir
    import concourse.tile as tile
    from concourse import bacc
    from concourse.bass import ds

    F32 = mybir.dt.float32
    BF = mybir.dt.bfloat16
    AF = mybir.ActivationFunctionType
    OP = mybir.AluOpType

    nc = bacc.Bacc("TRN2", target_bir_lowering=False, debug=False,
                   num_devices=NCORES)

    x0_d = nc.dram_tensor("x0t", [P, KC, PADT], BF, kind="ExternalInput")
    wx_d = nc.dram_tensor("wx", [3, P, KC, GC, P], BF, kind="ExternalInput")
    wh_d = nc.dram_tensor("wh", [3, P, KC, GC, P], BF, kind="ExternalInput")
    wt_d = nc.dram_tensor("wt", [2, P, 2 * KC, KC, P], BF, kind="ExternalInput")
    wc_d = nc.dram_tensor("wc", [2, P, KC, KC, P], BF, kind="ExternalInput")
    bias_d = nc.dram_tensor("bias", [3, P, GC], F32, kind="ExternalInput")
    bt_d = nc.dram_tensor("bt", [2, P, KC], F32, kind="ExternalInput")
    mask_d = nc.dram_tensor("mask2", [P, T * KC * BC], BF, kind="ExternalInput")
    out_d = nc.dram_tensor("outt", [P, KC, TB], F32, kind="ExternalOutput")

    with tile.TileContext(nc) as tc:
        with (
            tc.tile_pool(name="per", bufs=1) as per,
            tc.tile_pool(name="wpool", bufs=1) as wpool,
            tc.tile_pool(name="work", bufs=3) as work,
            tc.tile_pool(name="hwork", bufs=3) as hwork,
            tc.tile_pool(name="psb", bufs=4, space="PSUM") as psb,
            tc.tile_pool(name="psbig", bufs=3, space="PSUM") as psbig,
        ):
            buf1 = per.tile([P, KC, PADT], BF)
            buf2 = per.tile([P, KC, PADT], BF)
            buf3 = per.tile([P, KC, PADT], BF)
            xgt = per.tile([P, GC, TB], BF)
            biasb = per.tile([P, 3 * GC], F32)
            btb = per.tile([P, 2 * KC], F32)
            maskb = per.tile([P, T * KC * BC], BF)
            c = per.tile([P, KC, BC], F32)

            # initial loads
            nc.sync.dma_start(buf1[:], x0_d[:])
            for l in range(3):
                nc.sync.dma_start(biasb[:, l * GC:(l + 1) * GC], bias_d[l])
            for li in range(2):
                nc.sync.dma_start(btb[:, li * KC:(li + 1) * KC], bt_d[li])
            nc.sync.dma_start(maskb[:], mask_d[:])
            # zero pads of the two reusable buffers (buf1 pads come from host)
            for buf in (buf2, buf3):
                nc.vector.memset(buf[:, :, 0:PAD], 0.0)
                nc.vector.memset(buf[:, :, PAD + TB:], 0.0)

            def load_w(pool, src, shape, tag):
                t_ = pool.tile(shape, BF, tag=tag)
                nc.sync.dma_start(t_[:], src)
                return t_

            def phase_xg(l, xin, wxb):
                for gp in range(GC):
                    for n in range(NCH):
                        psum = psbig.tile([P, NSZ], F32, tag="big")
                        for k in range(KC):
                            nc.tensor.matmul(
                                psum[:], wxb[:, k, gp, :],
                                xin[:, k, PAD + n * NSZ:PAD + (n + 1) * NSZ],
                                start=(k == 0), stop=(k == KC - 1))
                        nc.scalar.activation(
                            xgt[:, gp, n * NSZ:(n + 1) * NSZ], psum[:],
                            AF.Identity, bias=biasb[:, l * GC + gp:l * GC + gp + 1])

            def mm_order():
                first_ks = list(range(KC // 2))
                rest_ks = list(range(KC // 2, KC))
                order = [(gp, k) for k in first_ks for gp in range(GC)]
                for g in range(NG):
                    order += [(gp, k)
                              for gp in range(g * GPG, (g + 1) * GPG)
                              for k in rest_ks]
                return order

            MM_ORDER = mm_order()

            GW = KC * BC  # mask columns per time step

            def phase_rec(l, yout, whb, rev):
                nc.vector.memset(c[:], 0.0)
                with tc.For_i(0, T // U, 1,
                              hint_engines=(mybir.EngineType.PE,)) as i:
                    for u in range(U):
                        if not rev:
                            # t = i*U + u
                            hcol = i * (BC * U) + u * BC            # PAD+(t-1)*BC
                            tcol = i * (BC * U) + u * BC            # t*BC
                            wcol = i * (BC * U) + PAD + u * BC      # PAD+t*BC
                            mcol = i * (GW * U) + u * GW            # t*GW
                        else:
                            # t = T-1 - (i*U + u)
                            hcol = i * (-BC * U) + PAD + (T - u) * BC
                            tcol = i * (-BC * U) + (T - 1 - u) * BC
                            wcol = i * (-BC * U) + PAD + (T - 1 - u) * BC
                            mcol = i * (-GW * U) + (T - 1 - u) * GW
                        pss = [psb.tile([P, GPG, BC], F32, tag="psb",
                                        name=f"ps{g_}")
                               for g_ in range(NG)]
                        for gp, k in MM_ORDER:
                            g, gl = divmod(gp, GPG)
                            # start=True clears has_written for the WHOLE bank,
                            # so only the first matmul into each psum tile may
                            # set it; other slices' first write (k==0) then
                            # overwrites (bit clear) and k>0 accumulates.
                            nc.tensor.matmul(
                                pss[g][:, gl, :], whb[:, k, gp, :],
                                yout[:, k, ds(hcol, BC)],
                                start=(k == 0 and gl == 0),
                                stop=(k == KC - 1),
                                skip_group_check=True)
                        for g in range(NG):
                            gsb = work.tile([P, 4 * S, BC], F32, tag="gsb")
                            nc.vector.tensor_tensor(
                                gsb[:], pss[g][:],
                                xgt[:, g * GPG:(g + 1) * GPG, ds(tcol, BC)],
                                OP.add)
                            tj = work.tile([P, S, BC], F32, tag="tj")
                            nc.scalar.activation(tj[:], gsb[:, 0:S, :], AF.Tanh)
                            sio = work.tile([P, 3 * S, BC], F32, tag="sio")
                            nc.scalar.activation(sio[:], gsb[:, S:4 * S, :],
                                                 AF.Sigmoid)
                            t1 = work.tile([P, S, BC], F32, tag="t1")
                            nc.vector.tensor_tensor(t1[:], sio[:, 0:S, :],
                                                    tj[:], OP.mult)
                            cg = c[:, g * S:(g + 1) * S, :]
                            nc.vector.tensor_tensor(cg, sio[:, S:2 * S, :],
                                                    cg, OP.mult)
                            nc.vector.tensor_tensor(cg, cg, t1[:], OP.add)
                            if rev:
                                msl = maskb[:, ds(mcol + g * S * BC, S * BC)]
                                nc.vector.tensor_tensor(
                                    cg, cg,
                                    msl.rearrange("p (a b) -> p a b", b=BC),
                                    OP.mult)
                            tct = work.tile([P, S, BC], F32, tag="tct")
                            nc.scalar.activation(tct[:], cg, AF.Tanh)
                            nc.vector.tensor_tensor(
                                yout[:, g * S:(g + 1) * S, ds(wcol, BC)],
                                sio[:, 2 * S:3 * S, :], tct[:], OP.mult)

            def phase_hw(li, y, x, out_sbuf, wtb, wcb):
                for gc_ in range(KC):
                    for n in range(NCH):
                        pt = psbig.tile([P, NSZ], F32, tag="big")
                        for k in range(KC):
                            nc.tensor.matmul(
                                pt[:], wtb[:, k, gc_, :],
                                y[:, k, PAD + n * NSZ - BC:PAD + (n + 1) * NSZ - BC],
                                start=(k == 0), stop=False)
                        for k in range(KC):
                            nc.tensor.matmul(
                                pt[:], wtb[:, KC + k, gc_, :],
                                x[:, k, PAD + n * NSZ:PAD + (n + 1) * NSZ],
                                start=False, stop=(k == KC - 1))
                        tg = hwork.tile([P, NSZ], BF, tag="tg")
                        nc.scalar.activation(
                            tg[:], pt[:], AF.Sigmoid,
                            bias=btb[:, li * KC + gc_:li * KC + gc_ + 1])
                        pc = psbig.tile([P, NSZ], F32, tag="big")
                        for k in range(KC):
                            nc.tensor.matmul(
                                pc[:], wcb[:, k, gc_, :],
                                x[:, k, PAD + n * NSZ:PAD + (n + 1) * NSZ],
                                start=(k == 0), stop=(k == KC - 1))
                        dt_ = hwork.tile([P, NSZ], F32, tag="dt")
                        nc.vector.tensor_tensor(
                            dt_[:], y[:, gc_, PAD + n * NSZ:PAD + (n + 1) * NSZ],
                            pc[:], OP.subtract)
                        nc.vector.tensor_tensor(dt_[:], dt_[:], tg[:], OP.mult)
                        if out_sbuf is not None:
                            nc.vector.tensor_tensor(
                                out_sbuf[:, gc_, PAD + n * NSZ:PAD + (n + 1) * NSZ],
                                dt_[:], pc[:], OP.add)
                        else:
                            st = hwork.tile([P, NSZ], F32, tag="st")
                            nc.vector.tensor_tensor(st[:], dt_[:], pc[:], OP.add)
                            nc.sync.dma_start(
                                out_d[:, gc_, n * NSZ:(n + 1) * NSZ], st[:])

            NL = int(os.environ.get("BASSLSTM_LAYERS", "3"))

            # ---- layer 1 (plain forward LSTM)
            wxb = load_w(wpool, wx_d[0], [P, KC, GC, P], "wx")
            whb = load_w(wpool, wh_d[0], [P, KC, GC, P], "wh")
            phase_xg(0, buf1, wxb)
            phase_rec(0, buf2, whb, rev=False)
            if NL == 1:
                for gc_ in range(KC):
                    for n in range(NCH):
                        st0 = hwork.tile([P, NSZ], F32, tag="st", name="st0")
                        nc.vector.tensor_copy(
                            st0[:],
                            buf2[:, gc_, PAD + n * NSZ:PAD + (n + 1) * NSZ])
                        nc.sync.dma_start(
                            out_d[:, gc_, n * NSZ:(n + 1) * NSZ], st0[:])
            else:
                wxb2 = load_w(wpool, wx_d[1], [P, KC, GC, P], "wx")
                wtb = load_w(wpool, wt_d[0], [P, 2 * KC, KC, P], "wt")
                wcb = load_w(wpool, wc_d[0], [P, KC, KC, P], "wc")
                whb2 = load_w(wpool, wh_d[1], [P, KC, GC, P], "wh")

                # ---- layer 2 (reversed LSTM + highway)
                phase_xg(1, buf2, wxb2)
                phase_rec(1, buf3, whb2, rev=True)
                if NL == 2:
                    phase_hw(0, buf3, buf2, None, wtb, wcb)
                else:
                    wxb3 = load_w(wpool, wx_d[2], [P, KC, GC, P], "wx")
                    whb3 = load_w(wpool, wh_d[2], [P, KC, GC, P], "wh")
                    phase_hw(0, buf3, buf2, buf1, wtb, wcb)
                    wtb2 = load_w(wpool, wt_d[1], [P, 2 * KC, KC, P], "wt")
                    wcb2 = load_w(wpool, wc_d[1], [P, KC, KC, P], "wc")

                    # ---- layer 3 (forward LSTM + highway -> DRAM)
                    phase_xg(2, buf1, wxb3)
                    phase_rec(2, buf2, whb3, rev=False)
                    phase_hw(1, buf2, buf1, None, wtb2, wcb2)

    nc.compile()
    return nc


def _program():
    global _PROG
    if _PROG is None:
        _PROG = _build_program()
    return _PROG


# ---------------------------------------------------------------- host side
def _prep_weights(inp):
    """Build the shared (replicated) weight arrays in device layout."""
    gtbl = _gprime_table()
    wx = np.zeros((3, P, KC, GC, P), np.float32)
    wh = np.zeros((3, P, KC, GC, P), np.float32)
    bias = np.zeros((3, P, GC), np.float32)
    for l, (wxn, whn, bn) in enumerate(
            [("Wx1", "Wh1", "b1"), ("Wx2", "Wh2", "b2"), ("Wx3", "Wh3", "b3")]):
        Wx = np.asarray(inp[wxn], np.float32)
        Wh = np.asarray(inp[whn], np.float32)
        b = np.asarray(inp[bn], np.float32)
        for gp, (go, m) in enumerate(gtbl):
            cs = go * H + m * P
            for k in range(KC):
                wx[l, :, k, gp, :] = Wx[k * P:(k + 1) * P, cs:cs + P]
                wh[l, :, k, gp, :] = Wh[k * P:(k + 1) * P, cs:cs + P]
            bias[l, :, gp] = b[cs:cs + P]
            if go == 2:  # forget gate: fold forget_bias = 1.0
                bias[l, :, gp] += 1.0
    wt = np.zeros((2, P, 2 * KC, KC, P), np.float32)
    wc = np.zeros((2, P, KC, KC, P), np.float32)
    bt = np.zeros((2, P, KC), np.float32)
    for li, (wtn, wcn, btn) in enumerate(
            [("Wt2", "Wc2", "bt2"), ("Wt3", "Wc3", "bt3")]):
        Wt = np.asarray(inp[wtn], np.float32)
        Wc = np.asarray(inp[wcn], np.float32)
        btv = np.asarray(inp[btn], np.float32)
        for gc_ in range(KC):
            cs = gc_ * P
            for k in range(2 * KC):
                wt[li, :, k, gc_, :] = Wt[k * P:(k + 1) * P, cs:cs + P]
            for k in range(KC):
                wc[li, :, k, gc_, :] = Wc[k * P:(k + 1) * P, cs:cs + P]
            bt[li, :, gc_] = btv[cs:cs + P]
    return (wx.astype(BF16), wh.astype(BF16), wt.astype(BF16),
            wc.astype(BF16), bias, bt)


def _host_prep(inputs):
    x = np.asarray(inputs["inputs"], np.float32)
    length = np.asarray(inputs["length"], np.int32)
    wx, wh, wt, wc, bias, bt = _prep_weights(inputs)
    in_maps = []
    for ci in range(NCORES):
        xc = x[ci * BC:(ci + 1) * BC, :T]          # [BC, T, D]
        arr = np.ascontiguousarray(xc.transpose(2, 1, 0))  # [D, T, BC]
        x0t = np.zeros((P, KC, PADT), BF16)
        x0t[:, :, PAD:PAD + TB] = (
            arr.reshape(KC, P, TB).transpose(1, 0, 2).astype(BF16))
        lc = length[ci * BC:(ci + 1) * BC]
        m = (np.arange(T)[:, None] < lc[None, :]).astype(np.float32)  # [T, BC]
        m4 = np.broadcast_to(m[:, None, :], (T, KC, BC))   # same for each chunk
        mask2 = np.ascontiguousarray(
            np.broadcast_to(m4.reshape(1, -1), (P, T * KC * BC))).astype(BF16)
        in_maps.append({
            "x0t": x0t,
            "wx": wx, "wh": wh, "wt": wt, "wc": wc,
            "bias": bias, "bt": bt,
            "mask2": mask2,
        })
    return in_maps


def _host_post(results, inputs):
    length = np.asarray(inputs["length"], np.int32)
    out = np.zeros((B, T, D), np.float32)
    for ci, res in enumerate(results):
        o = res["outt"]                      # [P, KC, TB]
        o = o.reshape(P, KC, T, BC).transpose(3, 2, 1, 0)  # [BC, T, KC, P]
        out[ci * BC:(ci + 1) * BC] = o.reshape(BC, T, D)
    tmask = np.arange(T)[None, :] < length[:, None]
    out *= tmask[:, :, None]
    return out


_RUNNER = None


def _make_runner():
    """Build a cached shard_map-jitted executable for the 8-core program.

    Modeled on concourse.bass2jax.run_bass_via_pjrt, but reusable across
    calls and able to take pre-placed (device-resident) inputs so pure
    execution can be timed without host->device transfer.
    """
    import jax
    import numpy as jnp_np  # noqa: F401
    import concourse.mybir as mybir
    from concourse import bass2jax
    from jax.sharding import Mesh, PartitionSpec
    from jax.experimental.shard_map import shard_map

    nc = _program()
    bass2jax.install_neuronx_cc_hook()

    partition_name = (nc.partition_id_tensor.name
                      if nc.partition_id_tensor else None)
    in_names, out_names, out_avals, zero_outs = [], [], [], []
    for alloc in nc.m.functions[0].allocations:
        if not isinstance(alloc, mybir.MemoryLocationSet):
            continue
        name = alloc.memorylocations[0].name
        if alloc.kind == "ExternalInput":
            if name != partition_name:
                in_names.append(name)
        elif alloc.kind == "ExternalOutput":
            shape = tuple(alloc.tensor_shape)
            dtype = mybir.dt.np(alloc.dtype)
            out_names.append(name)
            out_avals.append(jax.core.ShapedArray(shape, dtype))
            zero_outs.append(np.zeros(shape, dtype))
    n_params = len(in_names)
    all_names = in_names + out_names
    if partition_name is not None:
        all_names.append(partition_name)

    def _body(*args):
        operands = list(args)
        if partition_name is not None:
            operands.append(bass2jax.partition_id_tensor())
        outs = bass2jax._bass_exec_p.bind(
            *operands,
            out_avals=tuple(out_avals),
            in_names=tuple(all_names),
            out_names=tuple(out_names),
            lowering_input_output_aliases=(),
            sim_require_finite=True,
            sim_require_nnan=True,
            nc=nc,
        )
        return tuple(outs)

    devices = jax.devices()[:NCORES]
    mesh = Mesh(np.asarray(devices), ("core",))
    n_out = len(out_names)
    sharded = jax.jit(
        shard_map(_body, mesh=mesh,
                  in_specs=(PartitionSpec("core"),) * (n_params + n_out),
                  out_specs=(PartitionSpec("core"),) * n_out,
                  check_rep=False),
        keep_unused=True,
    )
    return {
        "fn": sharded, "in_names": in_names, "out_names": out_names,
        "zero_outs": zero_outs, "n_params": n_params,
    }


def _runner():
    global _RUNNER
    if _RUNNER is None:
        _RUNNER = _make_runner()
    return _RUNNER


def place_inputs(in_maps):
    """Concatenate per-core inputs on axis 0 (shard_map layout)."""
    r = _runner()
    concat = [np.concatenate([np.asarray(in_maps[c][n])
                              for c in range(NCORES)], axis=0)
              for n in r["in_names"]]
    concat += [np.zeros((NCORES * z.shape[0], *z.shape[1:]), z.dtype)
               for z in r["zero_outs"]]
    return concat


def exec_placed(placed):
    r = _runner()
    outs = r["fn"](*placed)
    outs = [o.block_until_ready() for o in outs]
    results = []
    for c in range(NCORES):
        m = {}
        for i, name in enumerate(r["out_names"]):
            z = r["zero_outs"][i]
            arr = np.asarray(outs[i])
            m[name] = arr[c * z.shape[0]:(c + 1) * z.shape[0]]
        results.append(m)
    return results


def run_device(in_maps):
    return exec_placed(place_inputs(in_maps))


def kernel(**inputs):
    in_maps = _host_prep(inputs)
    results = run_device(in_maps)
    return _host_post(results, inputs)
